# revision 16
# baseline (speedup 1.0000x reference)
"""Trainium2 Bass kernel for nn_EstNUNet (ConvNeXt U-Net, 8-way data parallel).

kernel(**inputs) takes FULL unsharded inputs (batch 8), shards the batch
across 8 NeuronCores (1 sample/core, weights replicated), runs one Bass
program per core via run_bass_kernel_spmd (axon/PJRT path), returns FULL
outputs as the same (img_NL, img_ND, curv_E) tuple the reference produces.

Layout: channels on partitions, spatial (H+6, W+6) zero-padded on the free
dim; activations DRAM-resident between blocks, streamed through SBUF in row
bands.  Depthwise 7x7 = 49 PSUM-accumulated diagonal matmuls with shifted
rhs views; pointwise convs = plain matmuls.  Matmul operands are retagged
float32r (reduced-precision fp32, full PE rate at N>=256).
"""

import sys

for _p in ("/root/.axon_site/_ro/trn_rl_repo", "/opt/trn_rl_repo"):
    if _p not in sys.path:
        sys.path.append(_p)

import numpy as np

import concourse.bass as bass  # noqa: F401
import concourse.mybir as mybir
from concourse import bacc
from concourse.tile import TileContext
from concourse.masks import make_identity

try:  # lift the stale 192 KiB/partition cap (224 phys, ~208 usable)
    import concourse.tile_utils as _tu
    if getattr(_tu, "max_sbuf_usage", None) == 192 * 1024:
        _tu.max_sbuf_usage = 204 * 1024
except Exception:
    pass

F32 = mybir.dt.float32
import os as _os
F32R = (mybir.dt.float32r if _os.environ.get("BASS_F32R", "0") == "1"
        else mybir.dt.float32)
ALU = mybir.AluOpType
ACTF = mybir.ActivationFunctionType
AXX = mybir.AxisListType.X

N_CORES = 8

STAGES = [("en1", 64, 256), ("en2", 128, 128), ("en3", 256, 64),
          ("body", 512, 32), ("de3", 256, 64), ("de2", 128, 128),
          ("de1", 64, 256)]

TAPS7 = [(dy, dx) for dy in range(7) for dx in range(7)]
TAPS3 = [(dy, dx) for dy in range(3) for dx in range(3)]

# rows per band for the streamed (C<=128) stages
RT = {256: 8, 128: 16}
MTG = 8  # max hidden-dim partition tiles per weight-load group


def _r(ap):
    return ap


class Net:
    def __init__(self, nc):
        self.nc = nc
        self.inputs = {}
        self.outputs = {}

    def dram_in(self, name, shape, dt=F32R):
        t = self.nc.dram_tensor(name, shape, dt, kind="ExternalInput")
        self.inputs[name] = t
        return t

    def dram_out(self, name, shape):
        t = self.nc.dram_tensor(name, shape, F32, kind="ExternalOutput")
        self.outputs[name] = t
        return t

    def dram_tmp(self, name, shape, dt=F32):
        return self.nc.dram_tensor(name, shape, dt)

    # ---------------- borders ----------------
    def pad_borders(self, dst, C, H, W, misc):
        """Zero all pad borders of a padded (C, H+6, W+6) DRAM activation."""
        nc = self.nc
        Cp = min(C, 128)
        Wp = W + 6
        z = misc.tile([128, 3 * 262], F32R, tag="zrow")
        nc.vector.memset(z[:], 0.0)
        d = dst[:].rearrange("(ct c) h w -> ct c h w", c=Cp)
        for ct in range(C // Cp):
            zv = z[0:Cp, 0:3 * Wp].rearrange("c (r w) -> c r w", w=Wp)
            nc.sync.dma_start(out=d[ct, :, 0:3, :], in_=zv)
            nc.sync.dma_start(out=d[ct, :, H + 3:H + 6, :], in_=zv)
            zc = z[0:Cp, 0:(H + 6) * 3].rearrange("c (h r) -> c h r", r=3)
            nc.sync.dma_start(out=d[ct, :, :, 0:3], in_=zc)
            nc.sync.dma_start(out=d[ct, :, :, W + 3:W + 6], in_=zc)

    # ---------------- block weights ----------------
    def _load_wk(self, wk, C, wp):
        nc = self.nc
        Cp = min(C, 128)
        Ct = C // Cp
        wk_sb = wp.tile([Cp, Ct * 49], F32R, tag="wk")
        nc.sync.dma_start(
            out=wk_sb[:].rearrange("c (ct t) -> c ct t", ct=Ct),
            in_=wk[:].rearrange("(ct c) t -> ct c t", c=Cp).transpose([1, 0, 2]))
        return wk_sb[:].rearrange("c (ct t) -> c ct t", ct=Ct)

    def _build_diags(self, wkv, ct, C, ident, dgp):
        nc = self.nc
        Cp = min(C, 128)
        dg = dgp.tile([Cp, 49 * Cp], F32R, tag="dg")
        dgv = dg[:].rearrange("c (t k) -> c t k", t=49)
        for t in range(49):
            nc.vector.tensor_scalar_mul(dgv[:, t, :], ident[0:Cp, 0:Cp],
                                        wkv[:, ct, t:t + 1].bitcast(F32))
        return dgv

    # ---------------- streamed ConvNeXt block (C <= 128) ----------------
    def block_streamed(self, src, dst, C, H, W, wk, w1, w2, ident, P):
        nc = self.nc
        assert C <= 128
        Wp = W + 6
        M1 = (4 * C) // 128
        rpc = 512 // W
        Rt = RT[H]
        nch = Rt // rpc
        sv = src[:]
        dv = dst[:]

        wkv = self._load_wk(wk, C, P.wp)
        w1_sb = P.wp.tile([C, 4 * C], F32R, tag="w1")
        nc.sync.dma_start(out=w1_sb[:], in_=w1[:])
        w2_sb = P.wp.tile([128, M1 * C], F32R, tag="w2")
        nc.sync.dma_start(
            out=w2_sb[:].rearrange("m (mt c) -> m mt c", mt=M1),
            in_=w2[:].rearrange("(mt m) c -> mt m c", m=128).transpose([1, 0, 2]))
        w2v = w2_sb[:].rearrange("m (mt c) -> m mt c", mt=M1)
        dgv = self._build_diags(wkv, 0, C, ident, P.dgp)

        for r0 in range(0, H, Rt):
            xb = P.xp.tile([C, (Rt + 6) * Wp], F32R, tag="xb")
            xbv = xb[:].rearrange("c (r w) -> c r w", w=Wp)
            nc.sync.dma_start(out=xbv, in_=sv[:, r0:r0 + Rt + 6, :])
            yb = P.yp.tile([C, Rt * W], F32R, tag="yb")
            ybv = yb[:].rearrange("c (r w) -> c r w", w=W)
            ob = P.op.tile([C, Rt * W], F32R, tag="ob")
            obv = ob[:].rearrange("c (r w) -> c r w", w=W)
            for ch in range(nch):
                ps = P.dwps.tile([C, 512], F32, space="PSUM", tag="ps")
                for t, (dy, dx) in enumerate(TAPS7):
                    rhs = xbv[:, ch * rpc + dy:ch * rpc + dy + rpc, dx:dx + W]
                    nc.tensor.matmul(ps[:], _r(dgv[:, t, :]), _r(rhs),
                                     start=(t == 0), stop=(t == 48))
                nc.scalar.activation(ybv[:, ch * rpc:(ch + 1) * rpc, :],
                                     ps[:].rearrange("c (r w) -> c r w", w=W),
                                     ACTF.Copy)
            for ch in range(nch):
                h1 = P.h1p.tile([128, M1 * 512], F32R, tag="h1")
                h1v = h1[:].rearrange("m (mt n) -> m mt n", mt=M1)
                for mt in range(M1):
                    ph = P.h1ps.tile([128, 512], F32, space="PSUM", tag="ph")
                    nc.tensor.matmul(ph[:],
                                     _r(w1_sb[:, mt * 128:(mt + 1) * 128]),
                                     _r(yb[:, ch * 512:(ch + 1) * 512]),
                                     start=True, stop=True)
                    nc.scalar.activation(h1v[:, mt, :], ph[:], ACTF.Relu)
                po = P.ops.tile([C, 512], F32, space="PSUM", tag="po0")
                for mt in range(M1):
                    nc.tensor.matmul(po[:], _r(w2v[:, mt, :]), _r(h1v[:, mt, :]),
                                     start=(mt == 0), stop=(mt == M1 - 1))
                xin = xbv[:, 3 + ch * rpc:3 + ch * rpc + rpc, 3:3 + W]
                nc.vector.tensor_add(
                    out=obv[:, ch * rpc:(ch + 1) * rpc, :],
                    in0=po[:].rearrange("c (r w) -> c r w", w=W), in1=xin)
            nc.sync.dma_start(out=dv[:, 3 + r0:3 + r0 + Rt, 3:3 + W], in_=obv)

    # ---------------- resident ConvNeXt block (C > 128, small H) --------
    def block_resident(self, src, dst, C, H, W, wk, w1, w2, ident, P):
        nc = self.nc
        Cp = 128
        Ct = C // Cp
        Wp = W + 6
        M1 = (4 * C) // 128
        G = (M1 + MTG - 1) // MTG
        rpc = 512 // W
        nch = (H * W) // 512
        sv = src[:].rearrange("(ct c) h w -> ct c h w", c=Cp)
        dv = dst[:].rearrange("(ct c) h w -> ct c h w", c=Cp)

        wkv = self._load_wk(wk, C, P.wp)
        yb = P.yp.tile([Cp, Ct * H * W], F32R, tag="ybr", bufs=1)
        ybv = yb[:].rearrange("c (ct n) -> c ct n", ct=Ct)
        for ct in range(Ct):
            xb = P.xp.tile([Cp, (H + 6) * Wp], F32R, tag="xb")
            xbv = xb[:].rearrange("c (r w) -> c r w", w=Wp)
            nc.sync.dma_start(out=xbv, in_=sv[ct])
            dgv = self._build_diags(wkv, ct, C, ident, P.dgp)
            for ch in range(nch):
                r = ch * rpc
                ps = P.dwps.tile([Cp, 512], F32, space="PSUM", tag="ps")
                for t, (dy, dx) in enumerate(TAPS7):
                    rhs = xbv[:, r + dy:r + dy + rpc, dx:dx + W]
                    nc.tensor.matmul(ps[:], _r(dgv[:, t, :]), _r(rhs),
                                     start=(t == 0), stop=(t == 48))
                nc.scalar.activation(ybv[:, ct, ch * 512:(ch + 1) * 512],
                                     ps[:], ACTF.Copy)

        w1d = w1[:].rearrange("(ct c) (mt m) -> ct c mt m", c=Cp, m=128)
        w2d = w2[:].rearrange("(mt m) (ct c) -> mt m ct c", m=128, c=Cp)
        for ch in range(nch):
            r = ch * rpc
            po = [P.ops.tile([Cp, 512], F32, space="PSUM", tag=f"po{ct}",
                              name=f"po{ct}_{ch}") for ct in range(Ct)]
            for g in range(G):
                mts = list(range(g * MTG, min(M1, (g + 1) * MTG)))
                w1_sb = P.wp.tile([Cp, Ct * len(mts) * 128], F32R, tag="w1")
                nc.sync.dma_start(
                    out=w1_sb[:],
                    in_=w1d[:, :, mts[0]:mts[-1] + 1, :].transpose([1, 0, 2, 3]))
                w1v = w1_sb[:].rearrange("c (ct g m) -> c ct g m",
                                         ct=Ct, g=len(mts))
                w2_sb = P.wp.tile([128, len(mts) * C], F32R, tag="w2")
                nc.sync.dma_start(
                    out=w2_sb[:],
                    in_=w2d[mts[0]:mts[-1] + 1].transpose([1, 0, 2, 3]))
                w2v = w2_sb[:].rearrange("m (g ct c) -> m g ct c",
                                         g=len(mts), ct=Ct)
                h1 = P.h1p.tile([128, len(mts) * 512], F32R, tag="h1")
                h1v = h1[:].rearrange("m (g n) -> m g n", g=len(mts))
                for i, mt in enumerate(mts):
                    ph = P.h1ps.tile([128, 512], F32, space="PSUM", tag="ph")
                    for ct in range(Ct):
                        nc.tensor.matmul(ph[:], _r(w1v[:, ct, i, :]),
                                         _r(ybv[:, ct, ch * 512:(ch + 1) * 512]),
                                         start=(ct == 0), stop=(ct == Ct - 1))
                    nc.scalar.activation(h1v[:, i, :], ph[:], ACTF.Relu)
                for ct in range(Ct):
                    for i, mt in enumerate(mts):
                        nc.tensor.matmul(po[ct][:], _r(w2v[:, i, ct, :]),
                                         _r(h1v[:, i, :]),
                                         start=(g == 0 and i == 0),
                                         stop=(g == G - 1 and i == len(mts) - 1))
            rx = P.yp.tile([Cp, Ct * 512], F32R, tag="yb")
            rxv = rx[:].rearrange("c (ct r w) -> c ct r w", ct=Ct, w=W)
            ob = P.op.tile([Cp, Ct * 512], F32R, tag="ob")
            obv = ob[:].rearrange("c (ct r w) -> c ct r w", ct=Ct, w=W)
            for ct in range(Ct):
                nc.sync.dma_start(out=rxv[:, ct],
                                  in_=sv[ct, :, 3 + r:3 + r + rpc, 3:3 + W])
                nc.vector.tensor_add(
                    out=obv[:, ct],
                    in0=po[ct][:].rearrange("c (r w) -> c r w", w=W),
                    in1=rxv[:, ct])
                nc.sync.dma_start(out=dv[ct, :, 3 + r:3 + r + rpc, 3:3 + W],
                                  in_=obv[:, ct])

    def block(self, src, dst, C, H, W, wk, w1, w2, ident, P):
        if C <= 128:
            self.block_streamed(src, dst, C, H, W, wk, w1, w2, ident, P)
        else:
            self.block_resident(src, dst, C, H, W, wk, w1, w2, ident, P)

    # ---------------- head ----------------
    def head(self, x0, dst, whead, P):
        nc = self.nc
        W = 256
        wh = P.wp.tile([1, 9 * 64], F32R, tag="w1")
        nc.sync.dma_start(out=wh[:], in_=whead[:])
        whv = wh[:].rearrange("p (t m) -> p t m", t=9)
        rpc, Rt = 2, 8
        for r0 in range(0, 256, Rt):
            xb = P.xp.tile([1, (Rt + 2) * 258], F32R, tag="xb")
            nc.vector.memset(xb[:], 0.0)
            xbv = xb[:].rearrange("p (r w) -> p r w", w=258)
            lo, hi = max(0, r0 - 1), min(256, r0 + Rt + 1)
            nc.sync.dma_start(out=xbv[:, lo - r0 + 1:hi - r0 + 1, 1:257],
                              in_=x0[lo:hi, :].unsqueeze(0))
            ob = P.op.tile([64, Rt * W], F32R, tag="ob")
            obv = ob[:].rearrange("c (r w) -> c r w", w=W)
            for ch in range(Rt // rpc):
                ps = P.dwps.tile([64, 512], F32, space="PSUM", tag="ps")
                for t, (dy, dx) in enumerate(TAPS3):
                    rhs = xbv[:, ch * rpc + dy:ch * rpc + dy + rpc, dx:dx + W]
                    nc.tensor.matmul(ps[:], _r(whv[:, t, :]), _r(rhs),
                                     start=(t == 0), stop=(t == 8))
                nc.scalar.activation(obv[:, ch * rpc:(ch + 1) * rpc, :],
                                     ps[:].rearrange("c (r w) -> c r w", w=W),
                                     ACTF.Copy)
            nc.sync.dma_start(out=dst[:, 3 + r0:3 + r0 + Rt, 3:3 + W], in_=obv)

    # ---------------- down conv (2x2 stride 2, C -> 2C) ----------------
    def down(self, src, dst, C, H, w4, P):
        nc = self.nc
        Cp = min(C, 128)
        Ct = C // Cp
        Mt = (2 * C) // 128
        W = H
        H2 = W2 = H // 2
        rpc = max(1, 512 // W2)
        Rt = min(rpc, H2)
        nper = (rpc * W2) // 512  # 1
        assert nper == 1 and Rt == rpc
        w_sb = P.wp.tile([Cp, 4 * Ct * 2 * C], F32R, tag="w1")
        nc.sync.dma_start(
            out=w_sb[:].rearrange("c (t ct m) -> c t ct m", t=4, ct=Ct),
            in_=w4[:].rearrange("t (ct c) m -> t ct c m", c=Cp)
            .transpose([2, 0, 1, 3]))
        wv = w_sb[:].rearrange("c (t ct m) -> c t ct m", t=4, ct=Ct)
        sv = src[:].rearrange("(ct c) h w -> ct c h w", c=Cp)
        dv = dst[:].rearrange("(mt m) h w -> mt m h w", m=128)
        for r0 in range(0, H2, Rt):
            xb = P.xp.tile([Cp, Ct * 2 * Rt * W], F32R, tag="xb")
            xbv = xb[:].rearrange("c (ct r a w b) -> c ct r a w b",
                                  ct=Ct, r=Rt, a=2, b=2)
            for ct in range(Ct):
                nc.sync.dma_start(
                    out=xb[:].rearrange("c (ct n) -> c ct n", ct=Ct)[:, ct],
                    in_=sv[ct, :, 3 + 2 * r0:3 + 2 * r0 + 2 * Rt, 3:3 + W])
            ob = P.op.tile([128, Mt * Rt * W2], F32R, tag="ob")
            obv = ob[:].rearrange("m (mt r w) -> m mt r w", mt=Mt, w=W2)
            for mt in range(Mt):
                ps = P.dwps.tile([128, 512], F32, space="PSUM", tag="ps")
                k = 0
                for t, (a, b) in enumerate([(0, 0), (0, 1), (1, 0), (1, 1)]):
                    for ct in range(Ct):
                        rhs = xbv[:, ct, :, a, :, b]
                        nc.tensor.matmul(
                            ps[:], _r(wv[:, t, ct, mt * 128:(mt + 1) * 128]),
                            _r(rhs), start=(k == 0), stop=(k == 4 * Ct - 1))
                        k += 1
                nc.scalar.activation(obv[:, mt],
                                     ps[:].rearrange("m (r w) -> m r w", w=W2),
                                     ACTF.Copy)
                nc.sync.dma_start(out=dv[mt, :, 3 + r0:3 + r0 + Rt, 3:3 + W2],
                                  in_=obv[:, mt])

    # ---------------- up conv (2x2 stride-2 transpose, C -> C/2) + skip --
    def up(self, src, dst, skip, C, H, w4, P):
        nc = self.nc
        Cp = min(C, 128)
        Ct = C // Cp
        Co = C // 2
        Cop = min(Co, 128)
        Mt = Co // Cop
        W = H
        W2 = 2 * W
        rpc = max(1, 512 // W)
        w_sb = P.wp.tile([Cp, 4 * Ct * Co], F32R, tag="w1")
        nc.sync.dma_start(
            out=w_sb[:].rearrange("c (t ct m) -> c t ct m", t=4, ct=Ct),
            in_=w4[:].rearrange("t (ct c) m -> t ct c m", c=Cp)
            .transpose([2, 0, 1, 3]))
        wv = w_sb[:].rearrange("c (t ct m) -> c t ct m", t=4, ct=Ct)
        sv = src[:].rearrange("(ct c) h w -> ct c h w", c=Cp)
        kv = skip[:].rearrange("(mt m) h w -> mt m h w", m=Cop)
        dv = dst[:].rearrange("(mt m) h w -> mt m h w", m=Cop)
        for r0 in range(0, H, rpc):
            xb = P.xp.tile([Cp, Ct * rpc * W], F32R, tag="xb")
            xbv = xb[:].rearrange("c (ct r w) -> c ct r w", ct=Ct, w=W)
            for ct in range(Ct):
                nc.sync.dma_start(out=xbv[:, ct],
                                  in_=sv[ct, :, 3 + r0:3 + r0 + rpc, 3:3 + W])
            for mt in range(Mt):
                kb = P.yp.tile([Cop, 2 * rpc * W2], F32R, tag="yb")
                kbv = kb[:].rearrange("m (r a w b) -> m r a w b", r=rpc, a=2, b=2)
                nc.sync.dma_start(
                    out=kb[:],
                    in_=kv[mt, :, 3 + 2 * r0:3 + 2 * r0 + 2 * rpc, 3:3 + W2])
                ob = P.op.tile([Cop, 2 * rpc * W2], F32R, tag="ob")
                obv = ob[:].rearrange("m (r a w b) -> m r a w b", r=rpc, a=2, b=2)
                for a in range(2):
                    for b in range(2):
                        ps = P.dwps.tile([Cop, 512], F32, space="PSUM", tag="ps")
                        for ct in range(Ct):
                            nc.tensor.matmul(
                                ps[:],
                                _r(wv[:, 2 * a + b, ct, mt * Cop:(mt + 1) * Cop]),
                                _r(xbv[:, ct]),
                                start=(ct == 0), stop=(ct == Ct - 1))
                        nc.vector.tensor_add(
                            out=obv[:, :, a, :, b],
                            in0=ps[:].rearrange("m (r w) -> m r w", w=W),
                            in1=kbv[:, :, a, :, b])
                nc.sync.dma_start(
                    out=dv[mt, :, 3 + 2 * r0:3 + 2 * r0 + 2 * rpc, 3:3 + W2],
                    in_=ob[:].rearrange("m (r w) -> m r w", w=W2))

    # ---------------- tail + NL/ND ----------------
    def tail(self, src, wtail, tail_out, out_nl, out_nd, P, misc):
        nc = self.nc
        W = 256
        wt = P.wp.tile([64, 18], F32R, tag="w1")
        nc.sync.dma_start(out=wt[:], in_=wtail[:])
        wtv = wt[:].rearrange("c (t m) -> c t m", t=9)
        sums = misc.tile([2, 128], F32, tag="sums")
        rpc, Rt = 2, 8
        for r0 in range(0, 256, Rt):
            xb = P.xp.tile([64, (Rt + 2) * 258], F32R, tag="xb")
            xbv = xb[:].rearrange("c (r w) -> c r w", w=258)
            nc.sync.dma_start(out=xbv,
                              in_=src[:, 2 + r0:2 + r0 + Rt + 2, 2:260])
            ob = P.op.tile([2, Rt * W], F32, tag="ob")
            obv = ob[:].rearrange("c (r w) -> c r w", w=W)
            for ch in range(Rt // rpc):
                ps = P.dwps.tile([2, 512], F32, space="PSUM", tag="ps")
                for t, (dy, dx) in enumerate(TAPS3):
                    rhs = xbv[:, ch * rpc + dy:ch * rpc + dy + rpc, dx:dx + W]
                    nc.tensor.matmul(ps[:], _r(wtv[:, t, :]), _r(rhs),
                                     start=(t == 0), stop=(t == 8))
                ci = r0 // rpc + ch
                nc.scalar.activation(obv[:, ch * rpc:(ch + 1) * rpc, :],
                                     ps[:].rearrange("c (r w) -> c r w", w=W),
                                     ACTF.Copy, accum_out=sums[:, ci:ci + 1])
            nc.sync.dma_start(out=tail_out[:, r0:r0 + Rt, :], in_=obv)
        tot_f = misc.tile([2, 1], F32, tag="totf")
        nc.vector.reduce_sum(tot_f[:], sums[:], axis=AXX)
        tot = misc.tile([2, 1], F32R, tag="tot")
        nc.vector.tensor_copy(out=tot[:], in_=tot_f[:])
        ones_row = misc.tile([1, 128], F32R, tag="ones_row")
        nc.vector.memset(ones_row[:], 1.0)
        pb = P.dwps.tile([128, 1], F32, space="PSUM", tag="ps")
        nc.tensor.matmul(pb[:], _r(ones_row[:]), _r(tot[0:1, :]),
                         start=True, stop=True)
        ones_big = misc.tile([128, 512], F32, tag="ones_big")
        nc.vector.memset(ones_big[:], 1.0)
        nl = misc.tile([128, 512], F32, tag="nl")
        nc.vector.tensor_scalar(nl[:], ones_big[:], pb[:], 1.0 / 65536.0,
                                op0=ALU.mult, op1=ALU.mult)
        nc.sync.dma_start(out=out_nl[:].rearrange("(p n) -> p n", p=128),
                          in_=nl[:])
        nc.sync.dma_start(out=out_nd[:],
                          in_=tail_out[1].rearrange("h w -> (h w)"))

    # ---------------- curvature ----------------
    def _extrap_pad_store(self, tiles, dst, pool, pfx):
        nc = self.nc
        pads = []
        for i in range(2):
            p = pool.tile([128, 258], F32, tag=f"pp{i}")
            nc.vector.tensor_copy(out=p[:, 1:257], in_=tiles[i][:])
            nc.vector.scalar_tensor_tensor(
                out=p[:, 0:1], in0=p[:, 1:2], scalar=2.0, in1=p[:, 2:3],
                op0=ALU.mult, op1=ALU.subtract)
            nc.vector.scalar_tensor_tensor(
                out=p[:, 257:258], in0=p[:, 256:257], scalar=2.0,
                in1=p[:, 255:256], op0=ALU.mult, op1=ALU.subtract)
            pads.append(p)
        # stage boundary rows on partition 0 (engine APs must not start at
        # high partitions), then extrapolate there
        sc = pool.tile([1, 3 * 258], F32, tag="sc")
        nc.sync.dma_start(out=sc[0:1, 0:258], in_=pads[0][1:2, :])
        nc.sync.dma_start(out=sc[0:1, 258:516], in_=pads[1][127:128, :])
        nc.sync.dma_start(out=sc[0:1, 516:774], in_=pads[1][126:127, :])
        er = pool.tile([1, 2 * 258], F32, tag="er")
        nc.vector.scalar_tensor_tensor(
            out=er[0:1, 0:258], in0=pads[0][0:1, :], scalar=2.0,
            in1=sc[0:1, 0:258], op0=ALU.mult, op1=ALU.subtract)
        nc.vector.scalar_tensor_tensor(
            out=er[0:1, 258:516], in0=sc[0:1, 258:516], scalar=2.0,
            in1=sc[0:1, 516:774], op0=ALU.mult, op1=ALU.subtract)
        nc.sync.dma_start(out=dst[0:1, :], in_=er[0:1, 0:258])
        nc.sync.dma_start(out=dst[1:129, :], in_=pads[0][:])
        nc.sync.dma_start(out=dst[129:257, :], in_=pads[1][:])
        nc.sync.dma_start(out=dst[257:258, :], in_=er[0:1, 258:516])

    def _rowdiff(self, srcp, i, pool, tag):
        """0.5 * (src[r+1] - src[r-1]) for 128 rows starting at 128*i."""
        nc = self.nc
        u = pool.tile([128, 258], F32, tag="cldu")
        nc.sync.dma_start(out=u[:], in_=srcp[128 * i:128 * i + 128, :])
        d = pool.tile([128, 258], F32, tag="cldd")
        nc.sync.dma_start(out=d[:], in_=srcp[128 * i + 2:128 * i + 130, :])
        o = pool.tile([128, 256], F32, tag=f"{tag}o")
        nc.vector.tensor_sub(out=o[:], in0=d[:, 1:257], in1=u[:, 1:257])
        nc.vector.tensor_scalar_mul(o[:], o[:], 0.5)
        return o

    def _coldiff(self, srcp, i, pool, tag):
        nc = self.nc
        c = pool.tile([128, 258], F32, tag="cldc")
        nc.sync.dma_start(out=c[:], in_=srcp[128 * i + 1:128 * i + 129, :])
        o = pool.tile([128, 256], F32, tag=f"{tag}o")
        nc.vector.tensor_sub(out=o[:], in0=c[:, 2:258], in1=c[:, 0:256])
        nc.vector.tensor_scalar_mul(o[:], o[:], 0.5)
        return o

    def curvature(self, tail_out, x0, out_curv, pool):
        nc = self.nc
        ep = self.dram_tmp("imgE_pad", (258, 258))
        uxp = self.dram_tmp("ux_pad", (258, 258))
        uyp = self.dram_tmp("uy_pad", (258, 258))
        et = []
        for i in range(2):
            a_t = pool.tile([128, 258], F32, tag="cldu", name="cva")
            a = a_t[:, 0:256]
            nc.sync.dma_start(out=a, in_=tail_out[1, 128 * i:128 * i + 128, :])
            b_t = pool.tile([128, 258], F32R, tag="cldd", name="cvb")
            b = b_t[:, 0:256]
            nc.sync.dma_start(out=b, in_=x0[128 * i:128 * i + 128, :])
            e = pool.tile([128, 256], F32, tag=f"cve{i}")
            nc.vector.tensor_add(out=e[:], in0=a, in1=b)
            et.append(e)
        self._extrap_pad_store(et, ep, pool, "e")
        ux_t = [self._coldiff(ep, i, pool, f"ux{i}") for i in range(2)]
        uy_t = [self._rowdiff(ep, i, pool, f"uy{i}") for i in range(2)]
        self._extrap_pad_store(ux_t, uxp, pool, "x")
        self._extrap_pad_store(uy_t, uyp, pool, "y")
        for i in range(2):
            uxx = self._coldiff(uxp, i, pool, "xx")
            uxy = self._rowdiff(uxp, i, pool, "xy")
            uyy = self._rowdiff(uyp, i, pool, "yy")
            ux, uy = ux_t[i], uy_t[i]
            den = pool.tile([128, 256], F32, tag="den")
            nc.vector.tensor_mul(out=den[:], in0=ux[:], in1=ux[:])
            t2 = pool.tile([128, 256], F32, tag="t2")
            nc.vector.tensor_mul(out=t2[:], in0=uy[:], in1=uy[:])
            nc.vector.tensor_add(out=den[:], in0=den[:], in1=t2[:])
            nc.vector.tensor_scalar_add(den[:], den[:], 1.0)
            nc.vector.tensor_mul(out=den[:], in0=den[:], in1=den[:])
            rec = pool.tile([128, 256], F32, tag="rec")
            nc.vector.reciprocal(rec[:], den[:])
            num = pool.tile([128, 256], F32, tag="num")
            nc.vector.tensor_mul(out=num[:], in0=uxx[:], in1=uyy[:])
            nc.vector.tensor_mul(out=t2[:], in0=uxy[:], in1=uxy[:])
            nc.vector.tensor_sub(out=num[:], in0=num[:], in1=t2[:])
            nc.vector.tensor_mul(out=num[:], in0=num[:], in1=rec[:])
            nc.sync.dma_start(out=out_curv[128 * i:128 * i + 128, :], in_=num[:])


class Pools:
    pass


def build_program():
    nc = bacc.Bacc("TRN2", target_bir_lowering=False, debug=False)
    with TileContext(nc) as tc:
        net = Net(nc)
        x0 = net.dram_in("x0", (256, 256))
        whead = net.dram_in("w_head9", (1, 9 * 64))
        wtail = net.dram_in("w_tail18", (64, 18))
        wdown = {i: net.dram_in(f"w_down{i}p",
                                (4, 64 * 2 ** (i - 1), 128 * 2 ** (i - 1)))
                 for i in (1, 2, 3)}
        wup = {i: net.dram_in(f"w_up{i}p",
                              (4, 128 * 2 ** (i - 1), 64 * 2 ** (i - 1)))
               for i in (1, 2, 3)}
        sw = {}
        for s, d, _h in STAGES:
            sw[s] = [(net.dram_in(f"{s}_dwk{i}", (d, 49)),
                      net.dram_in(f"{s}_w1T{i}", (d, 4 * d)),
                      net.dram_in(f"{s}_w2T{i}", (4 * d, d)))
                     for i in range(2)]
        acts = {}
        for lvl, (c, h) in enumerate([(64, 256), (128, 128),
                                      (256, 64), (512, 32)], 1):
            acts[lvl] = [net.dram_tmp(f"l{lvl}{ab}", (c, h + 6, h + 6), F32R)
                         for ab in "ab"]
        skips = {1: net.dram_tmp("skip1", (64, 262, 262), F32R),
                 2: net.dram_tmp("skip2", (128, 134, 134), F32R),
                 3: net.dram_tmp("skip3", (256, 70, 70), F32R)}
        tail_dram = net.dram_tmp("tail_out", (2, 256, 256))
        out_nl = net.dram_out("out_nl", (65536,))
        out_nd = net.dram_out("out_nd", (65536,))
        out_curv = net.dram_out("out_curv", (256, 256))

        with (
            tc.tile_pool(name="misc", bufs=1) as misc,
            tc.tile_pool(name="wp", bufs=1) as wp,
            tc.tile_pool(name="xp", bufs=2) as xp,
            tc.tile_pool(name="yp", bufs=2) as yp,
            tc.tile_pool(name="op", bufs=2) as op,
            tc.tile_pool(name="dgp", bufs=1) as dgp,
            tc.tile_pool(name="h1p", bufs=1) as h1p,
            tc.tile_pool(name="cvp", bufs=1) as cvp,
            tc.tile_pool(name="dwps", bufs=2, space="PSUM") as dwps,
            tc.tile_pool(name="h1ps", bufs=2, space="PSUM") as h1ps,
            tc.tile_pool(name="ops", bufs=1, space="PSUM") as ops,
        ):
            P = Pools()
            P.misc, P.wp, P.xp, P.yp, P.op = misc, wp, xp, yp, op
            P.dgp, P.h1p = dgp, h1p
            P.dwps, P.h1ps, P.ops = dwps, h1ps, ops
            ident = misc.tile([128, 128], F32, tag="ident")
            make_identity(nc, ident[:])

            def run_stage(s, d, h, src, pair):
                a, b = pair
                net.pad_borders(a, d, h, h, misc)
                if b is not src:
                    net.pad_borders(b, d, h, h, misc)
                cur = src
                for i in range(2):
                    dstb = a if cur is not a else b
                    wk, w1, w2 = sw[s][i]
                    net.block(cur, dstb, d, h, h, wk, w1, w2, ident, P)
                    cur = dstb
                return cur

            net.pad_borders(acts[1][0], 64, 256, 256, misc)
            net.head(x0, acts[1][0], whead, P)
            x = run_stage("en1", 64, 256, acts[1][0], (acts[1][1], acts[1][0]))
            nc.sync.dma_start(out=skips[1][:], in_=x[:])
            net.pad_borders(acts[2][0], 128, 128, 128, misc)
            net.down(x, acts[2][0], 64, 256, wdown[1], P)
            x = run_stage("en2", 128, 128, acts[2][0], (acts[2][1], acts[2][0]))
            nc.sync.dma_start(out=skips[2][:], in_=x[:])
            net.pad_borders(acts[3][0], 256, 64, 64, misc)
            net.down(x, acts[3][0], 128, 128, wdown[2], P)
            x = run_stage("en3", 256, 64, acts[3][0], (acts[3][1], acts[3][0]))
            nc.sync.dma_start(out=skips[3][:], in_=x[:])
            net.pad_borders(acts[4][0], 512, 32, 32, misc)
            net.down(x, acts[4][0], 256, 64, wdown[3], P)
            x = run_stage("body", 512, 32, acts[4][0], (acts[4][1], acts[4][0]))
            net.pad_borders(acts[3][0], 256, 64, 64, misc)
            net.up(x, acts[3][0], skips[3], 512, 32, wup[3], P)
            x = run_stage("de3", 256, 64, acts[3][0], (acts[3][1], acts[3][0]))
            net.pad_borders(acts[2][0], 128, 128, 128, misc)
            net.up(x, acts[2][0], skips[2], 256, 64, wup[2], P)
            x = run_stage("de2", 128, 128, acts[2][0], (acts[2][1], acts[2][0]))
            net.pad_borders(acts[1][0], 64, 256, 256, misc)
            net.up(x, acts[1][0], skips[1], 128, 128, wup[1], P)
            x = run_stage("de1", 64, 256, acts[1][0], (acts[1][1], acts[1][0]))
            net.tail(x, wtail, tail_dram, out_nl, out_nd, P, misc)
            net.curvature(tail_dram, x0, out_curv, cvp)
    nc.compile()
    return nc


def prep_weights(inputs):
    f = np.float32

    def c(a):
        return np.ascontiguousarray(a, dtype=f)

    out = {}
    out["w_head9"] = c(np.asarray(inputs["w_head"]).reshape(64, 9).T
                       .reshape(1, 9 * 64))
    wt = np.asarray(inputs["w_tail"])            # (2, 64, 3, 3)
    out["w_tail18"] = c(wt.transpose(1, 2, 3, 0).reshape(64, 18))
    for i in (1, 2, 3):
        wd = np.asarray(inputs[f"w_down{i}"])    # (2C, C, 2, 2)
        out[f"w_down{i}p"] = c(wd.transpose(2, 3, 1, 0)
                               .reshape(4, wd.shape[1], wd.shape[0]))
        wu = np.asarray(inputs[f"w_up{i}"])      # (Cin, Cout, 2, 2)
        out[f"w_up{i}p"] = c(wu.transpose(2, 3, 0, 1)
                             .reshape(4, wu.shape[0], wu.shape[1]))
    for s, d, _h in STAGES:
        dw = np.asarray(inputs[f"{s}_dw"])       # (2, d, 1, 7, 7)
        w1 = np.asarray(inputs[f"{s}_w1"])       # (2, 4d, d)
        w2 = np.asarray(inputs[f"{s}_w2"])       # (2, d, 4d)
        for i in range(2):
            out[f"{s}_dwk{i}"] = c(dw[i].reshape(d, 49))
            out[f"{s}_w1T{i}"] = c(w1[i].T)
            out[f"{s}_w2T{i}"] = c(w2[i].T)
    return out


_CACHE = {}


def _get_runner():
    if "run" in _CACHE:
        return _CACHE["run"]
    import jax
    from jax.sharding import Mesh, PartitionSpec
    from jax.experimental.shard_map import shard_map
    from concourse import bass2jax
    from concourse.bass2jax import _bass_exec_p, partition_id_tensor

    bass2jax.install_neuronx_cc_hook()
    nc = build_program()
    partition_name = (nc.partition_id_tensor.name
                      if nc.partition_id_tensor else None)
    in_names, out_names, out_avals = [], [], []
    for alloc in nc.m.functions[0].allocations:
        if not isinstance(alloc, mybir.MemoryLocationSet):
            continue
        name = alloc.memorylocations[0].name
        if alloc.kind == "ExternalInput":
            if name != partition_name:
                in_names.append(name)
        elif alloc.kind == "ExternalOutput":
            out_names.append(name)
            shape = tuple(alloc.tensor_shape)
            out_avals.append(jax.core.ShapedArray(shape, mybir.dt.np(alloc.dtype)))
    n_params = len(in_names)
    all_names = in_names + out_names
    if partition_name is not None:
        all_names = all_names + [partition_name]

    def _body(*args):
        operands = list(args)
        if partition_name is not None:
            operands.append(partition_id_tensor())
        outs = _bass_exec_p.bind(
            *operands,
            out_avals=tuple(out_avals),
            in_names=tuple(all_names),
            out_names=tuple(out_names),
            lowering_input_output_aliases=(),
            sim_require_finite=True,
            sim_require_nnan=True,
            nc=nc,
        )
        return tuple(outs)

    devices = jax.devices()[:N_CORES]
    mesh = Mesh(np.asarray(devices), ("core",))
    nio = n_params + len(out_names)
    sharded = jax.jit(shard_map(
        _body, mesh=mesh, in_specs=(PartitionSpec("core"),) * nio,
        out_specs=(PartitionSpec("core"),) * len(out_names), check_rep=False),
        keep_unused=True)
    zero_outs = [np.zeros((N_CORES * a.shape[0],) + a.shape[1:], a.dtype)
                 for a in out_avals]

    def run(in_maps):
        key = tuple(id(m.get("x0")) for m in in_maps)
        cached = _CACHE.get("dev_in")
        if cached is not None and cached[0] == key:
            concat = cached[1]
        else:
            from jax.sharding import NamedSharding
            sh = NamedSharding(mesh, PartitionSpec("core"))
            concat = [jax.device_put(
                np.concatenate([np.asarray(m[k]) for m in in_maps], axis=0), sh)
                for k in in_names]
            _CACHE["dev_in"] = (key, concat)
        outs = sharded(*concat, *zero_outs)
        res = []
        for ci in range(N_CORES):
            d = {}
            for i, name in enumerate(out_names):
                a = np.asarray(outs[i]).reshape((N_CORES,) + out_avals[i].shape)
                d[name] = a[ci]
            res.append(d)
        return res

    _CACHE["run"] = run
    return run


def kernel(**inputs):
    run = _get_runner()
    wmap = prep_weights(inputs)
    x0 = np.asarray(inputs["x0"], np.float32)
    in_maps = []
    for ci in range(N_CORES):
        m = dict(wmap)
        m["x0"] = np.ascontiguousarray(x0[ci, 0])
        in_maps.append(m)
    res = run(in_maps)
    nl = np.stack([res[ci]["out_nl"].reshape(1, 256, 256)
                   for ci in range(N_CORES)])
    nd = np.stack([res[ci]["out_nd"].reshape(1, 256, 256)
                   for ci in range(N_CORES)])
    cv = np.stack([res[ci]["out_curv"].reshape(1, 256, 256)
                   for ci in range(N_CORES)])
    return (nl, nd, cv)


if __name__ == "__main__":
    import time
    t0 = time.time()
    build_program()
    print("built ok in", time.time() - t0, "s")
